# revision 46
# baseline (speedup 1.0000x reference)
"""Trainium2 Bass kernel for the 5-layer dilated sparse-conv encoder.

Network (per batch): 1ch -> [3x3x3 dil1] -> 2ch -> [3x3x3 dil2] -> 2ch
-> [3x3x3 dil4] -> 2ch -> [3x3x3 dil2] -> 2ch -> [1x1x1] -> sigmoid,
with relu+occupancy-mask after each hidden conv and mask after sigmoid.

Sharding: 8 cores = 2 batches x 4 z-slabs of 48 planes. Each core gets a
66-plane input slab (z halo 9) and computes its 48 output planes with no
cross-core communication.

Per-core algorithm: fp8(e4m3) contraction over z on the TensorEngine.
Activations live in SBUF as [2ch*64 z-partitions, 202 y-rows, 200 x-cols]
(5-row / 4-col zero halos; stored z-window is [z0-8, z1+8)). A conv layer
is 5 PSUM-accumulated matmuls per 2-row output tile: 4 fp8 DoubleRow
matmuls that each fold TWO of the 9 (dy,dx) taps into the virtual K=256
contraction (the pair index is an extra AP dim whose stride is the
dy-delta in rows or dx-delta in cols; DoubleRow streams 2 fp8/cycle so a
pair costs the same as one plain tap), plus 1 plain fp8 matmul for the
center tap. The 3 dz taps and both channels fold into banded weight
matrices; L1 is zero-padded to K=128 so the PE HAM activity monitor
un-throttles the clock. Zero halos make every tap a full-rect stream
(junk pad cols land in PSUM and are never read; the outer 4+4 are
trimmed from the stream). relu+mask is one fused scalar_tensor_tensor
DVE op per psum tile, fp32 PSUM -> fp8 SBUF, emitted right behind each
center matmul so banks recycle early. L5 (1x1 conv + sigmoid + mask,
outputs remapped to a contiguous 96-partition block) is interleaved one
group behind L4 across 4 dedicated psum banks: ACT drains sigmoid from
PSUM, the mask-mult alternates DVE/GpSimd, and stores stream out per
8-row group. Masks are host-precomputed fp8 tensors; all input DMAs are
chunked so the pipeline starts as soon as the first rows land.
"""

import os
import sys

import numpy as np


def _ensure_import_path():
    for p in ("/opt/trn_rl_repo", "/root/.axon_site/_ro/trn_rl_repo"):
        if os.path.isdir(p) and p not in sys.path:
            sys.path.insert(0, p)


_ensure_import_path()

import ml_dtypes  # noqa: E402

import concourse.mybir as mybir  # noqa: E402
import concourse.tile as tile  # noqa: E402
from concourse import bacc, bass_utils  # noqa: E402

F8 = ml_dtypes.float8_e4m3  # matches mybir.dt.float8e4

B, D = 2, 192
ZS = 48  # z planes per core
HZ = 9  # input z halo
ZIN = ZS + 2 * HZ  # 66 input planes per core
YR = 202  # tile rows: 5 zero + 192 + 5 zero  (r = y + 5)
XW = 200  # tile cols: 4 zero + 192 + 4 zero  (c = x + 4)
NS = 2 * XW - 8  # moving free size per 2-row tile (outer pad trimmed)

# (dilation, valid out-z window in 64-coords) per conv layer
LAYERS = [(1, 0, 64), (2, 2, 62), (4, 6, 58), (2, 8, 56)]
V5 = (8, 56)

GRP = 4  # row-pair tiles per PSUM group (= conv psum depth)

# Tap schedule: slots 0-3 are DoubleRow pairs [(dy,dx) j=0, (dy,dx) j=1],
# slot 4 is the single center tap.
PAIRS = [
    ((-1, -1), (1, -1)),
    ((-1, 0), (1, 0)),
    ((-1, 1), (1, 1)),
    ((0, -1), (0, 1)),
]
CENTER = (0, 0)


def _build_bands(W1, W2, W3, W4, W5):
    """fp8 banded lhsT weight tensors, one [K, 10, 128] per conv layer
    (slot 2i+j holds pair i's tap j; slot 8 the center tap), plus b5."""
    Ws = [np.asarray(w, np.float32) for w in (W1, W2, W3, W4)]
    out = {}
    for li, (d, a, b) in enumerate(LAYERS):
        w = Ws[li]
        K = 128
        bb = np.zeros((K, 10, 128), np.float32)
        taps = [t for p in PAIRS for t in p] + [CENTER, (9, 9)]
        for s, (dy, dx) in enumerate(taps):
            if dy == 9:
                continue  # slot 9 unused (zero)
            zv = np.arange(a, b)
            for co in range(2):
                for dz in (-1, 0, 1):
                    if li == 0:
                        bb[zv + 1 + dz, s, co * 64 + zv] = w[
                            co, 0, dz + 1, dy + 1, dx + 1
                        ]
                    else:  # noqa
                        for ci in range(2):
                            bb[ci * 64 + zv + d * dz, s, co * 64 + zv] = w[
                                co, ci, dz + 1, dy + 1, dx + 1
                            ]
        out[f"b{li + 1}"] = bb.astype(F8)
    w5 = np.asarray(W5, np.float32)
    b5 = np.zeros((128, 96), np.float32)
    zv = np.arange(V5[0], V5[1])
    for co in range(2):
        for ci in range(2):
            b5[ci * 64 + zv, co * 48 + zv - V5[0]] = w5[co, ci, 0, 0, 0]
    out["b5"] = b5.astype(F8)
    return out


def _mut_ap(ap, dims, offset):
    """Return a copy of `ap` with free dims replaced by `dims`
    [(stride, size), ...] and element offset set to `offset`."""
    c = ap.copy()
    v = c.ap
    while len(v) > 1 + len(dims):
        v.pop()
    for i, (st, sz) in enumerate(dims):
        if 1 + i < len(v):
            v[1 + i] = (st, sz)
        else:
            v.append((st, sz))
    c.ap = v
    c.offset = offset
    return c


def build_program():
    f8 = mybir.dt.float8e4
    f32 = mybir.dt.float32
    DR = mybir.MatmulPerfMode.DoubleRow
    nc = bacc.Bacc("TRN2", target_bir_lowering=False, debug=False)

    xslab = nc.dram_tensor("xslab", [128, YR, XW], f8, kind="ExternalInput")
    b1d = nc.dram_tensor("b1", [128, 10, 128], f8, kind="ExternalInput")
    b2d = nc.dram_tensor("b2", [128, 10, 128], f8, kind="ExternalInput")
    b3d = nc.dram_tensor("b3", [128, 10, 128], f8, kind="ExternalInput")
    b4d = nc.dram_tensor("b4", [128, 10, 128], f8, kind="ExternalInput")
    b5d = nc.dram_tensor("b5", [128, 96], f8, kind="ExternalInput")
    maskd = nc.dram_tensor("maskd", [128, D, D], f8, kind="ExternalInput")
    mask5d = nc.dram_tensor("mask5d", [96, D, XW], f8, kind="ExternalInput")
    prob_o = nc.dram_tensor("prob_o", [ZS, D, D], f32, kind="ExternalOutput")
    regr_o = nc.dram_tensor("regr_o", [ZS, D, D], f32, kind="ExternalOutput")

    with tile.TileContext(nc) as tc:
        with (
            tc.tile_pool(name="wpool", bufs=1) as wp,
            tc.tile_pool(name="actA", bufs=1) as pa,
            tc.tile_pool(name="actB", bufs=1) as pb,
            tc.tile_pool(name="mkp", bufs=1) as mkp,
            tc.tile_pool(name="otp", bufs=6) as otp,
            tc.tile_pool(name="ps", bufs=4, space="PSUM") as ps,
            tc.tile_pool(name="ps5", bufs=2, space="PSUM") as ps5,
        ):
            b1t = wp.tile([128, 10, 128], f8)
            b2t = wp.tile([128, 10, 128], f8)
            b3t = wp.tile([128, 10, 128], f8)
            b4t = wp.tile([128, 10, 128], f8)
            b5t = wp.tile([128, 96], f8)
            nc.scalar.dma_start(b1t[:], b1d[:])
            for t, dram in ((b2t, b2d), (b3t, b3d), (b4t, b4d), (b5t, b5d)):
                nc.gpsimd.dma_start(t[:], dram[:])

            mk = mkp.tile([128, D, D], f8, tag="mk")
            mk5 = mkp.tile([96, D, XW], f8, tag="mk5", name="mk5")


            def act_tile(pool, tg, nm):
                return pool.tile([128, YR, XW], f8, tag=tg, name=nm)

            xt = act_tile(pa, "A", "xt")
            t1 = act_tile(pb, "B", "t1")

            # input slab, host-zeroed halos + zero parts 66:128 (so L1 can
            # run K=128, keeping the PE HAM activity monitor warm), DMA'd in
            # y-chunks so L1 can start early
            for r0, r1 in ((0, 16), (16, 80), (80, 144), (144, YR)):
                nc.sync.dma_start(xt[:, r0:r1, :], xslab[:, r0:r1, :])

            # zero t1's halos (t1/t3 buffer reuse keeps them zero)
            for tl, p0 in ((t1, 0),):
                nc.vector.memset(tl[p0:128, 0:5, :], 0.0)
                nc.vector.memset(tl[p0:128, 197:202, :], 0.0)
                nc.vector.memset(tl[p0:128, 5:197, 0:4], 0.0)
                nc.vector.memset(tl[p0:128, 5:197, 196:200], 0.0)

            # host-precomputed occupancy mask [2ch*64z, y, x], DMA'd in
            # y-chunks so the first epilogues don't wait on the whole volume
            for y0, y1 in ((0, 12), (12, 24), (24, 48), (48, 96), (96, 144), (144, D)):
                nc.scalar.dma_start(mk[:, y0:y1, :], maskd[:, y0:y1, :])
            for y0 in range(0, D, 96):
                nc.scalar.dma_start(
                    mk5[:, y0 : y0 + 96, :], mask5d[:, y0 : y0 + 96, :]
                )

            def l5_quad(t4, g0, q, ot):
                """Two L5 row-pairs into one 2-bank psum tile (each matmul
                stays within a bank), then ONE 4-row flat sigmoid drain --
                fewer, larger ACT ops."""
                acc = ps5.tile([96, 1024], f32, tag="psum5", name="acc5")
                for h in range(2):
                    y = g0 + 4 * q + 2 * h
                    nc.tensor.matmul(
                        acc[:, 512 * h : 512 * h + NS],
                        b5t[:, :],
                        _mut_ap(
                            t4[0:128, 0:2, 0:XW], [(1, NS)], (y + 5) * XW + 4
                        ),
                        start=True,
                        stop=True,
                    )
                pv = _mut_ap(acc[:, 0:1024], [(512, 2), (1, NS)], 0)
                ov = _mut_ap(
                    ot[0:96, 0:2, 0:XW], [(2 * XW, 2), (1, NS)], 4 * q * XW + 4
                )
                nc.scalar.activation(ov, pv, mybir.ActivationFunctionType.Sigmoid)

            def l5_finish(g0, ot, fine=False):
                rows = 2 * GRP
                if fine:
                    # trailing group: drain per row-pair so mult/DMA pipeline
                    # instead of serializing after the last matmul
                    for r in range(0, rows, 2):
                        nc.vector.tensor_tensor(
                            ot[:, r : r + 2, 4:196],
                            ot[:, r : r + 2, 4:196],
                            mk5[:, g0 + r : g0 + r + 2, 4:196],
                            op=mybir.AluOpType.mult,
                        )
                        nc.sync.dma_start(
                            prob_o[:, g0 + r : g0 + r + 2, :],
                            ot[0:48, r : r + 2, 4:196],
                        )
                        nc.sync.dma_start(
                            regr_o[:, g0 + r : g0 + r + 2, :],
                            ot[48:96, r : r + 2, 4:196],
                        )
                    return
                eng = nc.vector if (g0 // (2 * GRP)) % 2 == 0 else nc.gpsimd
                ov = _mut_ap(
                    ot[0:96, 0:2, 0:XW], [(2 * XW, rows // 2), (1, NS)], 4
                )
                mv = _mut_ap(
                    mk5[0:96, 0:2, 0:XW],
                    [(2 * XW, rows // 2), (1, NS)],
                    g0 * XW + 4,
                )
                eng.tensor_tensor(ov, ov, mv, op=mybir.AluOpType.mult)
                nc.sync.dma_start(
                    prob_o[:, g0 : g0 + rows, :], ot[0:48, 0:rows, 4:196]
                )
                nc.sync.dma_start(
                    regr_o[:, g0 : g0 + rows, :], ot[48:96, 0:rows, 4:196]
                )

            def conv_group(src, K, bt, d, dst, g0, l5_g0=None):
                """Conv MMs+epilogue for GRP row-pair tiles at rows g0..
                Epilogues interleave with the center matmuls so psum banks
                free up as early as possible. If l5_g0 is not None, the L5
                work (matmuls + sigmoid + mask + store) for the one-group-old
                rows l5_g0 is appended, so the ACT queue's lag never gates
                the PE."""
                accs = []
                for t in range(GRP):
                    accs.append(ps.tile([128, NS], f32, tag="psum", name="acc"))

                def acc_of(t):
                    return accs[t][:, 0:NS]
                # 4 DoubleRow pair matmuls + 1 center, weight-switch
                # amortized across the GRP tiles
                for i, ((dyA, dxA), (dyB, dxB)) in enumerate(PAIRS):
                    if dyA == dyB:  # x-pair: j stride along x
                        jst, r_d, c_d = 2 * d, 0, -d
                    else:  # y-pair: j stride 2d rows
                        jst, r_d, c_d = 2 * d * XW, -d, dxA * d
                    for t in range(GRP):
                        y = g0 + 2 * t
                        off = (y + 5 + r_d) * XW + c_d + 4
                        rhs = _mut_ap(
                            src[0:K, 0:2, 0:XW], [(jst, 2), (1, NS)], off
                        )
                        nc.tensor.matmul(
                            acc_of(t),
                            bt[0:K, 2 * i : 2 * i + 2, :],
                            rhs,
                            start=(i == 0),
                            stop=False,
                            perf_mode=DR,
                        )
                for t in range(GRP):
                    y = g0 + 2 * t
                    nc.tensor.matmul(
                        acc_of(t),
                        bt[0:K, 8, :],
                        _mut_ap(src[0:K, 0:2, 0:XW], [(1, NS)], (y + 5) * XW + 4),
                        start=False,
                        stop=True,
                    )
                    # epilogue right behind its center matmul:
                    # dst = relu(acc) * mask, fp32 PSUM -> fp8
                    pv = _mut_ap(accs[t][:, 0:NS], [(XW, 2), (1, D)], 0)
                    nc.vector.scalar_tensor_tensor(
                        dst[:, y + 5 : y + 7, 4:196],
                        pv,
                        0.0,
                        mk[:, y : y + 2, :],
                        op0=mybir.AluOpType.max,
                        op1=mybir.AluOpType.mult,
                    )
                if l5_g0 is not None:
                    ot = otp.tile([96, 2 * GRP, XW], f32, tag="ot", name="ot")
                    for q in range(GRP // 2):
                        l5_quad(dst, l5_g0, q, ot)
                    l5_finish(l5_g0, ot)

            t2 = act_tile(pa, "A", "t2")
            t3 = act_tile(pb, "B", "t3")
            t4 = act_tile(pa, "A", "t4")
            chain = (
                (xt, 128, b1t, 1, t1),
                (t1, 128, b2t, 2, t2),
                (t2, 128, b3t, 4, t3),
                (t3, 128, b4t, 2, t4),
            )
            for li, (src, K, bt, dil, dst) in enumerate(chain):
                for g0 in range(0, D, 2 * GRP):
                    l5g = g0 - 2 * GRP if li == 3 and g0 > 0 else None
                    conv_group(src[:], K, bt, dil, dst, g0, l5_g0=l5g)
            # trailing L5 for the last group
            g0 = D - 2 * GRP
            ot = otp.tile([96, 2 * GRP, XW], f32, tag="ot", name="ot")
            for q in range(GRP // 2):
                l5_quad(t4, g0, q, ot)
            l5_finish(g0, ot, fine=True)

    nc.compile()
    return nc


_prog_cache = {}


def make_in_maps(data, W1, W2, W3, W4, W5):
    bands = _build_bands(W1, W2, W3, W4, W5)
    data = np.asarray(data, np.float32)
    q = data.astype(F8)
    # preserve occupancy: nonzero values that underflow fp8 get the min
    # subnormal so the on-chip mask (q != 0) matches (data != 0)
    tiny = np.float32(2.0**-9) * np.where(data < 0, -1.0, 1.0).astype(np.float32)
    q = np.where((data != 0) & (q.astype(np.float32) == 0), tiny.astype(F8), q)
    dpad = np.zeros((B, D + 2 * HZ, YR, XW), F8)
    dpad[:, HZ : HZ + D, 5 : 5 + D, 4 : 4 + D] = q
    in_maps = []
    for c in range(8):
        bi, s = c // 4, c % 4
        canvas = np.zeros((128, YR, XW), F8)
        canvas[0:ZIN] = dpad[bi, s * ZS : s * ZS + ZIN]
        occ = (
            dpad[bi, s * ZS + 1 : s * ZS + 65, 5 : 5 + D, 4 : 4 + D]
            .astype(np.float32)
            != 0
        )
        m = np.concatenate([occ, occ], axis=0).astype(F8)
        o5 = occ[V5[0] : V5[1]]
        m5 = np.zeros((96, D, XW), F8)
        m5[:, :, 4:196] = np.concatenate([o5, o5], axis=0).astype(F8)
        in_maps.append(dict(xslab=canvas, maskd=m, mask5d=m5, **bands))
    return in_maps


def kernel(data, W1, W2, W3, W4, W5):
    _ensure_import_path()
    if "nc" not in _prog_cache:
        _prog_cache["nc"] = build_program()
    nc = _prog_cache["nc"]

    in_maps = make_in_maps(data, W1, W2, W3, W4, W5)
    res = bass_utils.run_bass_kernel_spmd(nc, in_maps, list(range(8))).results

    prob = np.zeros((B, 1, D, D, D), np.float32)
    regr = np.zeros((B, 1, D, D, D), np.float32)
    for c in range(8):
        bi, s = c // 4, c % 4
        prob[bi, 0, s * ZS : (s + 1) * ZS] = res[c]["prob_o"]
        regr[bi, 0, s * ZS : (s + 1) * ZS] = res[c]["regr_o"]
    return (prob, regr)


# revision 47
# speedup vs baseline: 1.0016x; 1.0016x over previous
"""Trainium2 Bass kernel for the 5-layer dilated sparse-conv encoder.

Network (per batch): 1ch -> [3x3x3 dil1] -> 2ch -> [3x3x3 dil2] -> 2ch
-> [3x3x3 dil4] -> 2ch -> [3x3x3 dil2] -> 2ch -> [1x1x1] -> sigmoid,
with relu+occupancy-mask after each hidden conv and mask after sigmoid.

Sharding: 8 cores = 2 batches x 4 z-slabs of 48 planes. Each core gets a
66-plane input slab (z halo 9) and computes its 48 output planes with no
cross-core communication.

Per-core algorithm: fp8(e4m3) contraction over z on the TensorEngine.
Activations live in SBUF as [2ch*64 z-partitions, 202 y-rows, 200 x-cols]
(5-row / 4-col zero halos; stored z-window is [z0-8, z1+8)). A conv layer
is 5 PSUM-accumulated matmuls per 2-row output tile: 4 fp8 DoubleRow
matmuls that each fold TWO of the 9 (dy,dx) taps into the virtual K=256
contraction (the pair index is an extra AP dim whose stride is the
dy-delta in rows or dx-delta in cols; DoubleRow streams 2 fp8/cycle so a
pair costs the same as one plain tap), plus 1 plain fp8 matmul for the
center tap. The 3 dz taps and both channels fold into banded weight
matrices; L1 is zero-padded to K=128 so the PE HAM activity monitor
un-throttles the clock. Zero halos make every tap a full-rect stream
(junk pad cols land in PSUM and are never read; the outer 4+4 are
trimmed from the stream). relu+mask is one fused scalar_tensor_tensor
DVE op per psum tile, fp32 PSUM -> fp8 SBUF, emitted right behind each
center matmul so banks recycle early. L5 (1x1 conv + sigmoid + mask,
outputs remapped to a contiguous 96-partition block) is interleaved one
group behind L4 across 4 dedicated psum banks: ACT drains sigmoid from
PSUM, the mask-mult alternates DVE/GpSimd, and stores stream out per
8-row group. Masks are host-precomputed fp8 tensors; all input DMAs are
chunked so the pipeline starts as soon as the first rows land.
"""

import os
import sys

import numpy as np


def _ensure_import_path():
    for p in ("/opt/trn_rl_repo", "/root/.axon_site/_ro/trn_rl_repo"):
        if os.path.isdir(p) and p not in sys.path:
            sys.path.insert(0, p)


_ensure_import_path()

import ml_dtypes  # noqa: E402

import concourse.mybir as mybir  # noqa: E402
import concourse.tile as tile  # noqa: E402
from concourse import bacc, bass_utils  # noqa: E402

F8 = ml_dtypes.float8_e4m3  # matches mybir.dt.float8e4

B, D = 2, 192
ZS = 48  # z planes per core
HZ = 9  # input z halo
ZIN = ZS + 2 * HZ  # 66 input planes per core
YR = 202  # tile rows: 5 zero + 192 + 5 zero  (r = y + 5)
XW = 200  # tile cols: 4 zero + 192 + 4 zero  (c = x + 4)
NS = 2 * XW - 8  # moving free size per 2-row tile (outer pad trimmed)

# (dilation, valid out-z window in 64-coords) per conv layer
LAYERS = [(1, 0, 64), (2, 2, 62), (4, 6, 58), (2, 8, 56)]
V5 = (8, 56)

GRP = 4  # row-pair tiles per PSUM group (= conv psum depth)

# Tap schedule: slots 0-3 are DoubleRow pairs [(dy,dx) j=0, (dy,dx) j=1],
# slot 4 is the single center tap.
PAIRS = [
    ((-1, -1), (1, -1)),
    ((-1, 0), (1, 0)),
    ((-1, 1), (1, 1)),
    ((0, -1), (0, 1)),
]
CENTER = (0, 0)


def _build_bands(W1, W2, W3, W4, W5):
    """fp8 banded lhsT weight tensors, one [K, 10, 128] per conv layer
    (slot 2i+j holds pair i's tap j; slot 8 the center tap), plus b5."""
    Ws = [np.asarray(w, np.float32) for w in (W1, W2, W3, W4)]
    out = {}
    for li, (d, a, b) in enumerate(LAYERS):
        w = Ws[li]
        K = 128
        bb = np.zeros((K, 10, 128), np.float32)
        taps = [t for p in PAIRS for t in p] + [CENTER, (9, 9)]
        for s, (dy, dx) in enumerate(taps):
            if dy == 9:
                continue  # slot 9 unused (zero)
            zv = np.arange(a, b)
            for co in range(2):
                for dz in (-1, 0, 1):
                    if li == 0:
                        bb[zv + 1 + dz, s, co * 64 + zv] = w[
                            co, 0, dz + 1, dy + 1, dx + 1
                        ]
                    else:  # noqa
                        for ci in range(2):
                            bb[ci * 64 + zv + d * dz, s, co * 64 + zv] = w[
                                co, ci, dz + 1, dy + 1, dx + 1
                            ]
        out[f"b{li + 1}"] = bb.astype(F8)
    w5 = np.asarray(W5, np.float32)
    b5 = np.zeros((128, 96), np.float32)
    zv = np.arange(V5[0], V5[1])
    for co in range(2):
        for ci in range(2):
            b5[ci * 64 + zv, co * 48 + zv - V5[0]] = w5[co, ci, 0, 0, 0]
    out["b5"] = b5.astype(F8)
    return out


def _mut_ap(ap, dims, offset):
    """Return a copy of `ap` with free dims replaced by `dims`
    [(stride, size), ...] and element offset set to `offset`."""
    c = ap.copy()
    v = c.ap
    while len(v) > 1 + len(dims):
        v.pop()
    for i, (st, sz) in enumerate(dims):
        if 1 + i < len(v):
            v[1 + i] = (st, sz)
        else:
            v.append((st, sz))
    c.ap = v
    c.offset = offset
    return c


def build_program():
    f8 = mybir.dt.float8e4
    f32 = mybir.dt.float32
    DR = mybir.MatmulPerfMode.DoubleRow
    nc = bacc.Bacc("TRN2", target_bir_lowering=False, debug=False)

    xslab = nc.dram_tensor("xslab", [128, YR, XW], f8, kind="ExternalInput")
    b1d = nc.dram_tensor("b1", [128, 10, 128], f8, kind="ExternalInput")
    b2d = nc.dram_tensor("b2", [128, 10, 128], f8, kind="ExternalInput")
    b3d = nc.dram_tensor("b3", [128, 10, 128], f8, kind="ExternalInput")
    b4d = nc.dram_tensor("b4", [128, 10, 128], f8, kind="ExternalInput")
    b5d = nc.dram_tensor("b5", [128, 96], f8, kind="ExternalInput")
    maskd = nc.dram_tensor("maskd", [128, D, D], f8, kind="ExternalInput")
    mask5d = nc.dram_tensor("mask5d", [96, D, D], f8, kind="ExternalInput")
    prob_o = nc.dram_tensor("prob_o", [ZS, D, D], f32, kind="ExternalOutput")
    regr_o = nc.dram_tensor("regr_o", [ZS, D, D], f32, kind="ExternalOutput")

    with tile.TileContext(nc) as tc:
        with (
            tc.tile_pool(name="wpool", bufs=1) as wp,
            tc.tile_pool(name="actA", bufs=1) as pa,
            tc.tile_pool(name="actB", bufs=1) as pb,
            tc.tile_pool(name="mkp", bufs=1) as mkp,
            tc.tile_pool(name="otp", bufs=6) as otp,
            tc.tile_pool(name="ps", bufs=4, space="PSUM") as ps,
            tc.tile_pool(name="ps5", bufs=4, space="PSUM") as ps5,
        ):
            b1t = wp.tile([128, 10, 128], f8)
            b2t = wp.tile([128, 10, 128], f8)
            b3t = wp.tile([128, 10, 128], f8)
            b4t = wp.tile([128, 10, 128], f8)
            b5t = wp.tile([128, 96], f8)
            nc.scalar.dma_start(b1t[:], b1d[:])
            for t, dram in ((b2t, b2d), (b3t, b3d), (b4t, b4d), (b5t, b5d)):
                nc.gpsimd.dma_start(t[:], dram[:])

            mk = mkp.tile([128, D, D], f8, tag="mk")
            mk5 = mkp.tile([96, D, D], f8, tag="mk5", name="mk5")


            def act_tile(pool, tg, nm):
                return pool.tile([128, YR, XW], f8, tag=tg, name=nm)

            xt = act_tile(pa, "A", "xt")
            t1 = act_tile(pb, "B", "t1")

            # input slab, host-zeroed halos + zero parts 66:128 (so L1 can
            # run K=128, keeping the PE HAM activity monitor warm), DMA'd in
            # y-chunks so L1 can start early
            for r0, r1 in ((0, 16), (16, 80), (80, 144), (144, YR)):
                nc.sync.dma_start(xt[:, r0:r1, :], xslab[:, r0:r1, :])

            # zero t1's halos (t1/t3 buffer reuse keeps them zero)
            for tl, p0 in ((t1, 0),):
                nc.vector.memset(tl[p0:128, 0:5, :], 0.0)
                nc.vector.memset(tl[p0:128, 197:202, :], 0.0)
                nc.vector.memset(tl[p0:128, 5:197, 0:4], 0.0)
                nc.vector.memset(tl[p0:128, 5:197, 196:200], 0.0)

            # host-precomputed occupancy mask [2ch*64z, y, x], DMA'd in
            # y-chunks so the first epilogues don't wait on the whole volume
            for y0, y1 in ((0, 12), (12, 24), (24, 48), (48, 96), (96, 144), (144, D)):
                nc.scalar.dma_start(mk[:, y0:y1, :], maskd[:, y0:y1, :])
            for y0 in range(0, D, 96):
                nc.scalar.dma_start(
                    mk5[:, y0 : y0 + 96, :], mask5d[:, y0 : y0 + 96, :]
                )

            def l5_pair(t4, g0, t, ot, pool=None):
                """One L5 row-pair: 1x1 conv matmul + sigmoid from PSUM."""
                y = g0 + 2 * t
                p = pool or ps5
                acc = p.tile(
                    [96, NS],
                    f32,
                    tag="psum" if p is ps else "psum5",
                    name="acc5",
                )
                nc.tensor.matmul(
                    acc[:, 0:NS],
                    b5t[:, :],
                    _mut_ap(t4[0:128, 0:2, 0:XW], [(1, NS)], (y + 5) * XW + 4),
                    start=True,
                    stop=True,
                )
                pv = _mut_ap(acc[:, 0:NS], [(XW, 2), (1, D)], 0)
                nc.scalar.activation(
                    ot[:, 2 * t : 2 * t + 2, :],
                    pv,
                    mybir.ActivationFunctionType.Sigmoid,
                )

            def l5_finish(g0, ot, fine=False):
                rows = 2 * GRP
                if fine:
                    # trailing group: drain per row-pair so mult/DMA pipeline
                    # instead of serializing after the last matmul
                    for r in range(0, rows, 2):
                        nc.vector.tensor_tensor(
                            ot[:, r : r + 2, :],
                            ot[:, r : r + 2, :],
                            mk5[:, g0 + r : g0 + r + 2, :],
                            op=mybir.AluOpType.mult,
                        )
                        nc.sync.dma_start(
                            prob_o[:, g0 + r : g0 + r + 2, :],
                            ot[0:48, r : r + 2, :],
                        )
                        nc.sync.dma_start(
                            regr_o[:, g0 + r : g0 + r + 2, :],
                            ot[48:96, r : r + 2, :],
                        )
                    return
                eng = nc.vector if (g0 // (2 * GRP)) % 2 == 0 else nc.gpsimd
                eng.tensor_tensor(
                    ot[:, 0:rows, :],
                    ot[:, 0:rows, :],
                    mk5[:, g0 : g0 + rows, :],
                    op=mybir.AluOpType.mult,
                )
                nc.sync.dma_start(prob_o[:, g0 : g0 + rows, :], ot[0:48, 0:rows, :])
                nc.sync.dma_start(
                    regr_o[:, g0 : g0 + rows, :], ot[48:96, 0:rows, :]
                )

            def conv_group(src, K, bt, d, dst, g0, l5_g0=None):
                """Conv MMs+epilogue for GRP row-pair tiles at rows g0..
                Epilogues interleave with the center matmuls so psum banks
                free up as early as possible. If l5_g0 is not None, the L5
                work (matmuls + sigmoid + mask + store) for the one-group-old
                rows l5_g0 is appended, so the ACT queue's lag never gates
                the PE."""
                accs = []
                for t in range(GRP):
                    accs.append(ps.tile([128, NS], f32, tag="psum", name="acc"))

                def acc_of(t):
                    return accs[t][:, 0:NS]
                # 4 DoubleRow pair matmuls + 1 center, weight-switch
                # amortized across the GRP tiles
                for i, ((dyA, dxA), (dyB, dxB)) in enumerate(PAIRS):
                    if dyA == dyB:  # x-pair: j stride along x
                        jst, r_d, c_d = 2 * d, 0, -d
                    else:  # y-pair: j stride 2d rows
                        jst, r_d, c_d = 2 * d * XW, -d, dxA * d
                    for t in range(GRP):
                        y = g0 + 2 * t
                        off = (y + 5 + r_d) * XW + c_d + 4
                        rhs = _mut_ap(
                            src[0:K, 0:2, 0:XW], [(jst, 2), (1, NS)], off
                        )
                        nc.tensor.matmul(
                            acc_of(t),
                            bt[0:K, 2 * i : 2 * i + 2, :],
                            rhs,
                            start=(i == 0),
                            stop=False,
                            perf_mode=DR,
                        )
                for t in range(GRP):
                    y = g0 + 2 * t
                    nc.tensor.matmul(
                        acc_of(t),
                        bt[0:K, 8, :],
                        _mut_ap(src[0:K, 0:2, 0:XW], [(1, NS)], (y + 5) * XW + 4),
                        start=False,
                        stop=True,
                    )
                    # epilogue right behind its center matmul:
                    # dst = relu(acc) * mask, fp32 PSUM -> fp8
                    pv = _mut_ap(accs[t][:, 0:NS], [(XW, 2), (1, D)], 0)
                    nc.vector.scalar_tensor_tensor(
                        dst[:, y + 5 : y + 7, 4:196],
                        pv,
                        0.0,
                        mk[:, y : y + 2, :],
                        op0=mybir.AluOpType.max,
                        op1=mybir.AluOpType.mult,
                    )
                if l5_g0 is not None:
                    ot = otp.tile([96, 2 * GRP, D], f32, tag="ot", name="ot")
                    for t in range(GRP):
                        l5_pair(dst, l5_g0, t, ot)
                    l5_finish(l5_g0, ot)

            t2 = act_tile(pa, "A", "t2")
            t3 = act_tile(pb, "B", "t3")
            t4 = act_tile(pa, "A", "t4")
            chain = (
                (xt, 128, b1t, 1, t1),
                (t1, 128, b2t, 2, t2),
                (t2, 128, b3t, 4, t3),
                (t3, 128, b4t, 2, t4),
            )
            for li, (src, K, bt, dil, dst) in enumerate(chain):
                for g0 in range(0, D, 2 * GRP):
                    l5g = g0 - 2 * GRP if li == 3 and g0 > 0 else None
                    conv_group(src[:], K, bt, dil, dst, g0, l5_g0=l5g)
            # trailing L5 for the last group
            g0 = D - 2 * GRP
            ot = otp.tile([96, 2 * GRP, D], f32, tag="ot", name="ot")
            for t in range(GRP):
                l5_pair(t4, g0, t, ot, pool=(ps if t % 2 else ps5))
                r = 2 * t
                nc.vector.tensor_tensor(
                    ot[:, r : r + 2, :],
                    ot[:, r : r + 2, :],
                    mk5[:, g0 + r : g0 + r + 2, :],
                    op=mybir.AluOpType.mult,
                )
                nc.sync.dma_start(
                    prob_o[:, g0 + r : g0 + r + 2, :], ot[0:48, r : r + 2, :]
                )
                nc.sync.dma_start(
                    regr_o[:, g0 + r : g0 + r + 2, :], ot[48:96, r : r + 2, :]
                )

    nc.compile()
    return nc


_prog_cache = {}


def make_in_maps(data, W1, W2, W3, W4, W5):
    bands = _build_bands(W1, W2, W3, W4, W5)
    data = np.asarray(data, np.float32)
    q = data.astype(F8)
    # preserve occupancy: nonzero values that underflow fp8 get the min
    # subnormal so the on-chip mask (q != 0) matches (data != 0)
    tiny = np.float32(2.0**-9) * np.where(data < 0, -1.0, 1.0).astype(np.float32)
    q = np.where((data != 0) & (q.astype(np.float32) == 0), tiny.astype(F8), q)
    dpad = np.zeros((B, D + 2 * HZ, YR, XW), F8)
    dpad[:, HZ : HZ + D, 5 : 5 + D, 4 : 4 + D] = q
    in_maps = []
    for c in range(8):
        bi, s = c // 4, c % 4
        canvas = np.zeros((128, YR, XW), F8)
        canvas[0:ZIN] = dpad[bi, s * ZS : s * ZS + ZIN]
        occ = (
            dpad[bi, s * ZS + 1 : s * ZS + 65, 5 : 5 + D, 4 : 4 + D]
            .astype(np.float32)
            != 0
        )
        m = np.concatenate([occ, occ], axis=0).astype(F8)
        o5 = occ[V5[0] : V5[1]]
        m5 = np.concatenate([o5, o5], axis=0).astype(F8)
        in_maps.append(dict(xslab=canvas, maskd=m, mask5d=m5, **bands))
    return in_maps


def kernel(data, W1, W2, W3, W4, W5):
    _ensure_import_path()
    if "nc" not in _prog_cache:
        _prog_cache["nc"] = build_program()
    nc = _prog_cache["nc"]

    in_maps = make_in_maps(data, W1, W2, W3, W4, W5)
    res = bass_utils.run_bass_kernel_spmd(nc, in_maps, list(range(8))).results

    prob = np.zeros((B, 1, D, D, D), np.float32)
    regr = np.zeros((B, 1, D, D, D), np.float32)
    for c in range(8):
        bi, s = c // 4, c % 4
        prob[bi, 0, s * ZS : (s + 1) * ZS] = res[c]["prob_o"]
        regr[bi, 0, s * ZS : (s + 1) * ZS] = res[c]["regr_o"]
    return (prob, regr)


# revision 48
# speedup vs baseline: 1.0104x; 1.0088x over previous
"""Trainium2 Bass kernel for the 5-layer dilated sparse-conv encoder.

Network (per batch): 1ch -> [3x3x3 dil1] -> 2ch -> [3x3x3 dil2] -> 2ch
-> [3x3x3 dil4] -> 2ch -> [3x3x3 dil2] -> 2ch -> [1x1x1] -> sigmoid,
with relu+occupancy-mask after each hidden conv and mask after sigmoid.

Sharding: 8 cores = 2 batches x 4 z-slabs of 48 planes. Each core gets a
66-plane input slab (z halo 9) and computes its 48 output planes with no
cross-core communication.

Per-core algorithm: fp8(e4m3) contraction over z on the TensorEngine.
Activations live in SBUF as [2ch*64 z-partitions, 202 y-rows, 200 x-cols]
(5-row / 4-col zero halos; stored z-window is [z0-8, z1+8)). A conv layer
is 5 PSUM-accumulated matmuls per 2-row output tile: 4 fp8 DoubleRow
matmuls that each fold TWO of the 9 (dy,dx) taps into the virtual K=256
contraction (the pair index is an extra AP dim whose stride is the
dy-delta in rows or dx-delta in cols; DoubleRow streams 2 fp8/cycle so a
pair costs the same as one plain tap), plus 1 plain fp8 matmul for the
center tap. The 3 dz taps and both channels fold into banded weight
matrices; L1 is zero-padded to K=128 so the PE HAM activity monitor
un-throttles the clock. Zero halos make every tap a full-rect stream
(junk pad cols land in PSUM and are never read; the outer 4+4 are
trimmed from the stream). relu+mask is one fused scalar_tensor_tensor
DVE op per psum tile, fp32 PSUM -> fp8 SBUF, emitted right behind each
center matmul so banks recycle early. L5 (1x1 conv + sigmoid + mask,
outputs remapped to a contiguous 96-partition block) is interleaved one
group behind L4 across 4 dedicated psum banks: ACT drains sigmoid from
PSUM, the mask-mult alternates DVE/GpSimd, and stores stream out per
8-row group. Masks are host-precomputed fp8 tensors; all input DMAs are
chunked so the pipeline starts as soon as the first rows land.
"""

import os
import sys

import numpy as np


def _ensure_import_path():
    for p in ("/opt/trn_rl_repo", "/root/.axon_site/_ro/trn_rl_repo"):
        if os.path.isdir(p) and p not in sys.path:
            sys.path.insert(0, p)


_ensure_import_path()

import ml_dtypes  # noqa: E402

import concourse.mybir as mybir  # noqa: E402
import concourse.tile as tile  # noqa: E402
from concourse import bacc, bass_utils  # noqa: E402

F8 = ml_dtypes.float8_e4m3  # matches mybir.dt.float8e4

B, D = 2, 192
ZS = 48  # z planes per core
HZ = 9  # input z halo
ZIN = ZS + 2 * HZ  # 66 input planes per core
YR = 202  # tile rows: 5 zero + 192 + 5 zero  (r = y + 5)
XW = 200  # tile cols: 4 zero + 192 + 4 zero  (c = x + 4)
NS = 2 * XW - 8  # moving free size per 2-row tile (outer pad trimmed)

# (dilation, valid out-z window in 64-coords) per conv layer
LAYERS = [(1, 0, 64), (2, 2, 62), (4, 6, 58), (2, 8, 56)]
V5 = (8, 56)

GRP = 4  # row-pair tiles per PSUM group (= conv psum depth)

# Tap schedule: slots 0-3 are DoubleRow pairs [(dy,dx) j=0, (dy,dx) j=1],
# slot 4 is the single center tap.
PAIRS = [
    ((-1, -1), (1, -1)),
    ((-1, 0), (1, 0)),
    ((-1, 1), (1, 1)),
    ((0, -1), (0, 1)),
]
CENTER = (0, 0)


def _build_bands(W1, W2, W3, W4, W5):
    """fp8 banded lhsT weight tensors, one [K, 10, 128] per conv layer
    (slot 2i+j holds pair i's tap j; slot 8 the center tap), plus b5."""
    Ws = [np.asarray(w, np.float32) for w in (W1, W2, W3, W4)]
    out = {}
    for li, (d, a, b) in enumerate(LAYERS):
        w = Ws[li]
        K = 128
        bb = np.zeros((K, 10, 128), np.float32)
        taps = [t for p in PAIRS for t in p] + [CENTER, (9, 9)]
        for s, (dy, dx) in enumerate(taps):
            if dy == 9:
                continue  # slot 9 unused (zero)
            zv = np.arange(a, b)
            for co in range(2):
                for dz in (-1, 0, 1):
                    if li == 0:
                        bb[zv + 1 + dz, s, co * 64 + zv] = w[
                            co, 0, dz + 1, dy + 1, dx + 1
                        ]
                    else:  # noqa
                        for ci in range(2):
                            bb[ci * 64 + zv + d * dz, s, co * 64 + zv] = w[
                                co, ci, dz + 1, dy + 1, dx + 1
                            ]
        out[f"b{li + 1}"] = bb.astype(F8)
    w5 = np.asarray(W5, np.float32)
    b5 = np.zeros((128, 96), np.float32)
    zv = np.arange(V5[0], V5[1])
    for co in range(2):
        for ci in range(2):
            b5[ci * 64 + zv, co * 48 + zv - V5[0]] = w5[co, ci, 0, 0, 0]
    out["b5"] = b5.astype(F8)
    return out


def _mut_ap(ap, dims, offset):
    """Return a copy of `ap` with free dims replaced by `dims`
    [(stride, size), ...] and element offset set to `offset`."""
    c = ap.copy()
    v = c.ap
    while len(v) > 1 + len(dims):
        v.pop()
    for i, (st, sz) in enumerate(dims):
        if 1 + i < len(v):
            v[1 + i] = (st, sz)
        else:
            v.append((st, sz))
    c.ap = v
    c.offset = offset
    return c


def build_program():
    f8 = mybir.dt.float8e4
    f32 = mybir.dt.float32
    DR = mybir.MatmulPerfMode.DoubleRow
    nc = bacc.Bacc("TRN2", target_bir_lowering=False, debug=False)

    xslab = nc.dram_tensor("xslab", [128, YR, XW], f8, kind="ExternalInput")
    b1d = nc.dram_tensor("b1", [128, 10, 128], f8, kind="ExternalInput")
    b2d = nc.dram_tensor("b2", [128, 10, 128], f8, kind="ExternalInput")
    b3d = nc.dram_tensor("b3", [128, 10, 128], f8, kind="ExternalInput")
    b4d = nc.dram_tensor("b4", [128, 10, 128], f8, kind="ExternalInput")
    b5d = nc.dram_tensor("b5", [128, 96], f8, kind="ExternalInput")
    maskd = nc.dram_tensor("maskd", [128, D, D], f8, kind="ExternalInput")
    mask5d = nc.dram_tensor("mask5d", [96, D, D], f8, kind="ExternalInput")
    prob_o = nc.dram_tensor("prob_o", [ZS, D, D], f32, kind="ExternalOutput")
    regr_o = nc.dram_tensor("regr_o", [ZS, D, D], f32, kind="ExternalOutput")

    with tile.TileContext(nc) as tc:
        with (
            tc.tile_pool(name="wpool", bufs=1) as wp,
            tc.tile_pool(name="actA", bufs=1) as pa,
            tc.tile_pool(name="actB", bufs=1) as pb,
            tc.tile_pool(name="mkp", bufs=1) as mkp,
            tc.tile_pool(name="otp", bufs=6) as otp,
            tc.tile_pool(name="ps", bufs=4, space="PSUM") as ps,
            tc.tile_pool(name="ps5", bufs=4, space="PSUM") as ps5,
        ):
            b1t = wp.tile([128, 10, 128], f8)
            b2t = wp.tile([128, 10, 128], f8)
            b3t = wp.tile([128, 10, 128], f8)
            b4t = wp.tile([128, 10, 128], f8)
            b5t = wp.tile([128, 96], f8)
            nc.scalar.dma_start(b1t[:], b1d[:])
            for t, dram in ((b2t, b2d), (b3t, b3d), (b4t, b4d), (b5t, b5d)):
                nc.gpsimd.dma_start(t[:], dram[:])

            mk = mkp.tile([128, D, D], f8, tag="mk")
            mk5 = mkp.tile([96, D, D], f8, tag="mk5", name="mk5")

            # dependency-free warm-up matmuls: ~4us of PE busy-time starting
            # the moment the queues come up, so the HAM clock gate is open
            # (2.4 GHz) before L1's first real matmul ~12us in
            dummy = mkp.tile([128, 512], f8, tag="dummy", name="dummy")
            nc.gpsimd.memset(dummy[:], 0.0)
            for w in range(12):
                wacc = ps.tile([128, NS], f32, tag="psum", name="wacc")
                nc.tensor.matmul(
                    wacc[:, 0:NS],
                    dummy[:, 0:128],
                    dummy[:, 0:NS],
                    start=True,
                    stop=True,
                )


            def act_tile(pool, tg, nm):
                return pool.tile([128, YR, XW], f8, tag=tg, name=nm)

            xt = act_tile(pa, "A", "xt")
            t1 = act_tile(pb, "B", "t1")

            # input slab, host-zeroed halos + zero parts 66:128 (so L1 can
            # run K=128, keeping the PE HAM activity monitor warm), DMA'd in
            # y-chunks so L1 can start early
            for r0, r1 in ((0, 16), (16, 80), (80, 144), (144, YR)):
                nc.sync.dma_start(xt[:, r0:r1, :], xslab[:, r0:r1, :])

            # zero t1's halos (t1/t3 buffer reuse keeps them zero)
            for tl, p0 in ((t1, 0),):
                nc.vector.memset(tl[p0:128, 0:5, :], 0.0)
                nc.vector.memset(tl[p0:128, 197:202, :], 0.0)
                nc.vector.memset(tl[p0:128, 5:197, 0:4], 0.0)
                nc.vector.memset(tl[p0:128, 5:197, 196:200], 0.0)

            # host-precomputed occupancy mask [2ch*64z, y, x], DMA'd in
            # y-chunks so the first epilogues don't wait on the whole volume
            for y0, y1 in ((0, 12), (12, 24), (24, 48), (48, 96), (96, 144), (144, D)):
                nc.scalar.dma_start(mk[:, y0:y1, :], maskd[:, y0:y1, :])
            for y0 in range(0, D, 96):
                nc.scalar.dma_start(
                    mk5[:, y0 : y0 + 96, :], mask5d[:, y0 : y0 + 96, :]
                )

            def l5_pair(t4, g0, t, ot, pool=None):
                """One L5 row-pair: 1x1 conv matmul + sigmoid from PSUM."""
                y = g0 + 2 * t
                p = pool or ps5
                acc = p.tile(
                    [96, NS],
                    f32,
                    tag="psum" if p is ps else "psum5",
                    name="acc5",
                )
                nc.tensor.matmul(
                    acc[:, 0:NS],
                    b5t[:, :],
                    _mut_ap(t4[0:128, 0:2, 0:XW], [(1, NS)], (y + 5) * XW + 4),
                    start=True,
                    stop=True,
                )
                pv = _mut_ap(acc[:, 0:NS], [(XW, 2), (1, D)], 0)
                nc.scalar.activation(
                    ot[:, 2 * t : 2 * t + 2, :],
                    pv,
                    mybir.ActivationFunctionType.Sigmoid,
                )

            def l5_finish(g0, ot, fine=False):
                rows = 2 * GRP
                if fine:
                    # trailing group: drain per row-pair so mult/DMA pipeline
                    # instead of serializing after the last matmul
                    for r in range(0, rows, 2):
                        nc.vector.tensor_tensor(
                            ot[:, r : r + 2, :],
                            ot[:, r : r + 2, :],
                            mk5[:, g0 + r : g0 + r + 2, :],
                            op=mybir.AluOpType.mult,
                        )
                        nc.sync.dma_start(
                            prob_o[:, g0 + r : g0 + r + 2, :],
                            ot[0:48, r : r + 2, :],
                        )
                        nc.sync.dma_start(
                            regr_o[:, g0 + r : g0 + r + 2, :],
                            ot[48:96, r : r + 2, :],
                        )
                    return
                eng = nc.vector if (g0 // (2 * GRP)) % 2 == 0 else nc.gpsimd
                eng.tensor_tensor(
                    ot[:, 0:rows, :],
                    ot[:, 0:rows, :],
                    mk5[:, g0 : g0 + rows, :],
                    op=mybir.AluOpType.mult,
                )
                nc.sync.dma_start(prob_o[:, g0 : g0 + rows, :], ot[0:48, 0:rows, :])
                nc.sync.dma_start(
                    regr_o[:, g0 : g0 + rows, :], ot[48:96, 0:rows, :]
                )

            def conv_group(src, K, bt, d, dst, g0, l5_g0=None):
                """Conv MMs+epilogue for GRP row-pair tiles at rows g0..
                Epilogues interleave with the center matmuls so psum banks
                free up as early as possible. If l5_g0 is not None, the L5
                work (matmuls + sigmoid + mask + store) for the one-group-old
                rows l5_g0 is appended, so the ACT queue's lag never gates
                the PE."""
                accs = []
                for t in range(GRP):
                    accs.append(ps.tile([128, NS], f32, tag="psum", name="acc"))

                def acc_of(t):
                    return accs[t][:, 0:NS]
                # 4 DoubleRow pair matmuls + 1 center, weight-switch
                # amortized across the GRP tiles
                for i, ((dyA, dxA), (dyB, dxB)) in enumerate(PAIRS):
                    if dyA == dyB:  # x-pair: j stride along x
                        jst, r_d, c_d = 2 * d, 0, -d
                    else:  # y-pair: j stride 2d rows
                        jst, r_d, c_d = 2 * d * XW, -d, dxA * d
                    for t in range(GRP):
                        y = g0 + 2 * t
                        off = (y + 5 + r_d) * XW + c_d + 4
                        rhs = _mut_ap(
                            src[0:K, 0:2, 0:XW], [(jst, 2), (1, NS)], off
                        )
                        nc.tensor.matmul(
                            acc_of(t),
                            bt[0:K, 2 * i : 2 * i + 2, :],
                            rhs,
                            start=(i == 0),
                            stop=False,
                            perf_mode=DR,
                        )
                for t in range(GRP):
                    y = g0 + 2 * t
                    nc.tensor.matmul(
                        acc_of(t),
                        bt[0:K, 8, :],
                        _mut_ap(src[0:K, 0:2, 0:XW], [(1, NS)], (y + 5) * XW + 4),
                        start=False,
                        stop=True,
                    )
                    # epilogue right behind its center matmul:
                    # dst = relu(acc) * mask, fp32 PSUM -> fp8
                    pv = _mut_ap(accs[t][:, 0:NS], [(XW, 2), (1, D)], 0)
                    nc.vector.scalar_tensor_tensor(
                        dst[:, y + 5 : y + 7, 4:196],
                        pv,
                        0.0,
                        mk[:, y : y + 2, :],
                        op0=mybir.AluOpType.max,
                        op1=mybir.AluOpType.mult,
                    )
                if l5_g0 is not None:
                    ot = otp.tile([96, 2 * GRP, D], f32, tag="ot", name="ot")
                    for t in range(GRP):
                        l5_pair(dst, l5_g0, t, ot)
                    l5_finish(l5_g0, ot)

            t2 = act_tile(pa, "A", "t2")
            t3 = act_tile(pb, "B", "t3")
            t4 = act_tile(pa, "A", "t4")
            chain = (
                (xt, 128, b1t, 1, t1),
                (t1, 128, b2t, 2, t2),
                (t2, 128, b3t, 4, t3),
                (t3, 128, b4t, 2, t4),
            )
            for li, (src, K, bt, dil, dst) in enumerate(chain):
                for g0 in range(0, D, 2 * GRP):
                    l5g = g0 - 2 * GRP if li == 3 and g0 > 0 else None
                    conv_group(src[:], K, bt, dil, dst, g0, l5_g0=l5g)
            # trailing L5 for the last group
            g0 = D - 2 * GRP
            ot = otp.tile([96, 2 * GRP, D], f32, tag="ot", name="ot")
            for t in range(GRP):
                l5_pair(t4, g0, t, ot, pool=(ps if t % 2 else ps5))
                r = 2 * t
                nc.vector.tensor_tensor(
                    ot[:, r : r + 2, :],
                    ot[:, r : r + 2, :],
                    mk5[:, g0 + r : g0 + r + 2, :],
                    op=mybir.AluOpType.mult,
                )
                nc.sync.dma_start(
                    prob_o[:, g0 + r : g0 + r + 2, :], ot[0:48, r : r + 2, :]
                )
                nc.sync.dma_start(
                    regr_o[:, g0 + r : g0 + r + 2, :], ot[48:96, r : r + 2, :]
                )

    nc.compile()
    return nc


_prog_cache = {}


def make_in_maps(data, W1, W2, W3, W4, W5):
    bands = _build_bands(W1, W2, W3, W4, W5)
    data = np.asarray(data, np.float32)
    q = data.astype(F8)
    # preserve occupancy: nonzero values that underflow fp8 get the min
    # subnormal so the on-chip mask (q != 0) matches (data != 0)
    tiny = np.float32(2.0**-9) * np.where(data < 0, -1.0, 1.0).astype(np.float32)
    q = np.where((data != 0) & (q.astype(np.float32) == 0), tiny.astype(F8), q)
    dpad = np.zeros((B, D + 2 * HZ, YR, XW), F8)
    dpad[:, HZ : HZ + D, 5 : 5 + D, 4 : 4 + D] = q
    in_maps = []
    for c in range(8):
        bi, s = c // 4, c % 4
        canvas = np.zeros((128, YR, XW), F8)
        canvas[0:ZIN] = dpad[bi, s * ZS : s * ZS + ZIN]
        occ = (
            dpad[bi, s * ZS + 1 : s * ZS + 65, 5 : 5 + D, 4 : 4 + D]
            .astype(np.float32)
            != 0
        )
        m = np.concatenate([occ, occ], axis=0).astype(F8)
        o5 = occ[V5[0] : V5[1]]
        m5 = np.concatenate([o5, o5], axis=0).astype(F8)
        in_maps.append(dict(xslab=canvas, maskd=m, mask5d=m5, **bands))
    return in_maps


def kernel(data, W1, W2, W3, W4, W5):
    _ensure_import_path()
    if "nc" not in _prog_cache:
        _prog_cache["nc"] = build_program()
    nc = _prog_cache["nc"]

    in_maps = make_in_maps(data, W1, W2, W3, W4, W5)
    res = bass_utils.run_bass_kernel_spmd(nc, in_maps, list(range(8))).results

    prob = np.zeros((B, 1, D, D, D), np.float32)
    regr = np.zeros((B, 1, D, D, D), np.float32)
    for c in range(8):
        bi, s = c // 4, c % 4
        prob[bi, 0, s * ZS : (s + 1) * ZS] = res[c]["prob_o"]
        regr[bi, 0, s * ZS : (s + 1) * ZS] = res[c]["regr_o"]
    return (prob, regr)
